# revision 26
# baseline (speedup 1.0000x reference)
"""BlockXDiag (tri-diagonal block matrix × batch, periodic corners) on 8
Trainium2 NeuronCores.

Math (per reference): out_i = x_{i-1} @ A_i.T + x_i @ Wd_i.T + x_{i+1} @ Wu_i.T
for block-rows i in [0, 64), block size P=256, batch B=4096, with periodic
corner terms (x_63 @ Wtr.T into out_0, x_0 @ Wbl.T into out_63).

Sharding: output block-rows are split 8-per-core (weights split across
cores, which keeps per-core weight traffic small and lets each core stream
only its 10-block x halo). Everything device-side runs in bf16 (inputs cast
on host, output staged bf16 and upcast on host) so the kernel is
tensor-engine-bound: 768 matmuls/core x 216 ns = 166 us vs a 41 MB/core
HBM footprint (~115 us). Inputs are staged host-side as partition-major
slabs per batch-tile so every DMA is 20 KiB-contiguous per partition.

Device kernel per core: out.T[q, b] tiles [128, 512] accumulated in PSUM
over 6 matmuls (3 source blocks x 2 k-halves); x loads on the SP HW DMA
queue, weight loads (per-li chunks) + output stores (per-li, 256 KB) on
the Activation HW DMA queue; ~44 junk matmuls on a zeroed tile pre-warm
the PE HAM clock gate so the real stream runs at 2.4 GHz from the start.
Measured: 184,890 ns (baseline 235,304/245,723 ns), rel err 3.5e-3.
"""
import numpy as np
import ml_dtypes

import concourse.bass as bass
import concourse.mybir as mybir
from concourse.tile import TileContext
from concourse.vector_clock import ScopedClock
from concourse.bass_utils import run_bass_kernel_spmd

M, P, B = 64, 256, 4096
NCORES = 8
BPC = M // NCORES          # output blocks per core: 8
NHALO = BPC + 2            # x blocks needed per core: 10
ROWS = NHALO * P           # x^T rows per core: 2560
BT = 512                   # batch-tile (matmul moving free dim)
NBT = B // BT              # 8

MODE = "bf16"              # "f32" | "f32r" | "bf16"
TRACE = False              # set by test harness to profile
REPEATS = 1                # extra timed executions (test harness only)
LAST_EXEC_NS = None
ALL_EXEC_NS = None

_DT = {
    "f32": (mybir.dt.float32, np.float32),
    "f32r": (mybir.dt.float32r, np.float32),
    "bf16": (mybir.dt.bfloat16, ml_dtypes.bfloat16),
}


# ---------------------------------------------------------------------------
# Workarounds for the pinned walrus build's 1-wait-per-instruction cap.
# Tile's tail drain stuffs every outstanding sem wait onto one Drain, and
# self-loading fp32/fp32r matmuls can carry >1 wait with no Ldweights to
# spill to. Split both across extra same-engine instructions.
def _patched_drain_and_barrier(self, tick_clock, wait_clock):
    drain_inst = self.nc.sync.drain()
    wait_clock.add_sem_waits(
        drain_inst.ins, ScopedClock({None: tick_clock.global_clock})
    )
    si = drain_inst.ins.sync_info
    waits = list(si.on_wait)
    if len(waits) > 1:
        drain_inst.ins.sync_info = mybir.SyncInfo(
            on_wait=[waits[0]], on_update=list(si.on_update)
        )
        for w in waits[1:]:
            d2 = self.nc.sync.drain()
            d2.ins.sync_info = mybir.SyncInfo(on_wait=[w], on_update=[])
    self.nc.all_engine_barrier()
    assert self.sems is not None
    popped = self.nc._tile_sem_poison_stack.pop()
    assert popped is self._sem_poison
    self.nc.clear_and_free_semaphores(list(self.sems.allocated().values()))
    self.nc.all_engine_barrier()


def _apply_tile_patch():
    TileContext._drain_and_barrier = _patched_drain_and_barrier


def _install_profile_shim():
    """Make trace=True work in this container: provide the missing
    antenv.axon_hooks module (NTFF capture via ctypes into libaxon_pjrt.so)
    and skip the bucket upload of artifacts."""
    import sys, types, ctypes, contextlib
    import concourse.bass_utils as bu
    bu.upload_artifacts = lambda tmpdir: tmpdir
    try:
        from antenv.axon_hooks import get_axon_ntff_profile_hook  # noqa
        return
    except ImportError:
        pass
    so_path = "/opt/axon/libaxon_pjrt.so"
    lib = ctypes.CDLL(so_path)
    if not hasattr(lib, "axon_start_nrt_profile"):
        return
    lib.axon_start_nrt_profile.argtypes = [
        ctypes.POINTER(ctypes.c_int64), ctypes.c_size_t]
    lib.axon_start_nrt_profile.restype = ctypes.c_int64
    lib.axon_stop_nrt_profile.argtypes = [ctypes.c_char_p]
    lib.axon_stop_nrt_profile.restype = ctypes.c_int64

    @contextlib.contextmanager
    def _hook(output_dir, device_ids):
        import jax
        jax.devices()
        if device_ids:
            ids = (ctypes.c_int64 * len(device_ids))(*device_ids)
            rc = lib.axon_start_nrt_profile(ids, len(device_ids))
        else:
            rc = lib.axon_start_nrt_profile(None, 0)
        if rc != 0:
            raise RuntimeError(f"axon_start_nrt_profile rc={rc}")
        try:
            yield
        finally:
            n = lib.axon_stop_nrt_profile(str(output_dir).encode())
            print(f"profile: {n} file(s) written to {output_dir}")

    mod = types.ModuleType("antenv.axon_hooks")
    mod.get_axon_ntff_profile_hook = lambda: _hook
    mod.set_axon_ntff_profile_hook = lambda h: None
    sys.modules["antenv.axon_hooks"] = mod
    import antenv
    antenv.axon_hooks = mod


def _hoist_excess_waits(nc):
    """Any non-EventSemaphore instruction may carry at most 1 sem wait on
    this walrus build; move extras onto inserted same-engine NoOps."""
    for fn in nc.m.functions:
        for bb in fn.blocks:
            insts = bb.instructions
            newlist = []
            changed = False
            for inst in insts:
                si = inst.sync_info
                cap = 2 if isinstance(inst, mybir.InstEventSemaphore) else 1
                if si is not None and len(si.on_wait) > cap:
                    waits = list(si.on_wait)
                    for i, w in enumerate(waits[cap:]):
                        newlist.append(mybir.InstNoOp(
                            name=f"{inst.name}_waitnop{i}",
                            engine=inst.engine,
                            bass_nofuse=True,
                            sync_info=mybir.SyncInfo(on_wait=[w], on_update=[]),
                        ))
                    inst.sync_info = mybir.SyncInfo(
                        on_wait=waits[:cap], on_update=list(si.on_update))
                    changed = True
                newlist.append(inst)
            if changed:
                insts.clear()
                insts.extend(newlist)


# ---------------------------------------------------------------------------
def _build_nc(mode):
    dt_in, _ = _DT[mode]
    f32 = mybir.dt.float32
    # Stage the output in bf16 (host upcasts): halves HBM write traffic;
    # quantization of the final result adds <2e-3 rel err vs the 2e-2 gate.
    dt_out = mybir.dt.bfloat16 if mode == "bf16" else f32
    nc = bass.Bass()
    # Inputs are pre-tiled host-side into partition-major slabs so every DMA
    # transfer is long-contiguous per partition (the naive x^T layout gave
    # 1 KB rows in bf16, which halved effective DMA throughput).
    xT_d = nc.dram_tensor("xT", [NBT, 128, NHALO * 2, BT], dt_in,
                          kind="ExternalInput")
    w_d = nc.dram_tensor("w", [BPC, 128, 6, P], dt_in, kind="ExternalInput")
    o_d = nc.dram_tensor("o", [NBT, BPC, 128, 2, BT], dt_out,
                         kind="ExternalOutput")

    with TileContext(nc) as tc:
        with tc.tile_pool(name="wpool", bufs=1) as wpool, \
             tc.tile_pool(name="xpool", bufs=3) as xpool, \
             tc.tile_pool(name="opool", bufs=8) as opool, \
             tc.tile_pool(name="pspool", bufs=8, space="PSUM") as pspool:
            # PE warm-up: junk matmuls on a zeroed scratch tile start the HAM
            # activity window ~2us before the first x chunk lands, so the
            # real MM stream runs at 2.4 GHz almost from its first issue.
            wa = wpool.tile([128, 128], dt_in, tag="warm")
            nc.gpsimd.memset(wa, 0)
            wps = pspool.tile([128, BT], f32, tag="ps")
            NWARM = 44
            for i in range(NWARM):
                nc.tensor.matmul(wps[0:64, 0:128], wa[:, 0:64], wa[:, 0:128],
                                 start=(i == 0), stop=(i == NWARM - 1))

            w_sb = wpool.tile([128, BPC * 3 * 2, P], dt_in)
            # Weights go on the Activation-engine HW DMA queue (so they do
            # not sit in front of the x stream on the SP queue), in per-li
            # chunks so li=k's weights land just ahead of its matmuls.
            for li in range(BPC):
                nc.scalar.dma_start(
                    out=w_sb[:, li * 6:(li + 1) * 6, :],
                    in_=w_d[li])

            for bt in range(NBT):
                xt = xpool.tile([128, NHALO * 2, BT], dt_in, tag="x")
                if bt == 0:
                    # head split: 1-block chunks so li=0 starts after ~0.25 MB
                    for c in range(NHALO):
                        nc.sync.dma_start(
                            out=xt[:, c * 2:(c + 1) * 2, :],
                            in_=xT_d[0][:, c * 2:(c + 1) * 2, :],
                        )
                elif bt <= 2:
                    # the head window is bandwidth-tight: chunk these tiles
                    # too so matmuls wait on single blocks, not 2.6 MB tiles
                    for c in range(NHALO // 2):
                        nc.sync.dma_start(
                            out=xt[:, c * 4:(c + 1) * 4, :],
                            in_=xT_d[bt][:, c * 4:(c + 1) * 4, :],
                        )
                else:
                    nc.sync.dma_start(out=xt, in_=xT_d[bt])
                for li in range(BPC):       # fine-grained 256 KB out stores
                    ot = opool.tile([128, 2, BT], dt_out, tag="o")
                    if bt == 0 and li < 2:
                        # head: qh-innermost so only 1 new x block is needed
                        # per 4 matmuls while the first chunks stream in
                        ps0 = pspool.tile([128, BT], f32, tag="ps")
                        ps1 = pspool.tile([128, BT], f32, tag="ps")
                        pss = (ps0, ps1)
                        for s in range(3):
                            for kh in range(2):
                                for qh in range(2):
                                    nc.tensor.matmul(
                                        pss[qh],
                                        w_sb[:, (li * 3 + s) * 2 + kh,
                                             qh * 128:(qh + 1) * 128],
                                        xt[:, (li + s) * 2 + kh, :],
                                        start=(s == 0 and kh == 0),
                                        stop=(s == 2 and kh == 1),
                                    )
                        for qh in range(2):
                            nc.vector.tensor_copy(
                                out=ot[:, qh, :], in_=pss[qh])
                    else:
                        for qh in range(2):
                            ps = pspool.tile([128, BT], f32, tag="ps")
                            for s in range(3):
                                for kh in range(2):
                                    nc.tensor.matmul(
                                        ps,
                                        w_sb[:, (li * 3 + s) * 2 + kh,
                                             qh * 128:(qh + 1) * 128],
                                        xt[:, (li + s) * 2 + kh, :],
                                        start=(s == 0 and kh == 0),
                                        stop=(s == 2 and kh == 1),
                                    )
                            nc.vector.tensor_copy(
                                out=ot[:, qh, :], in_=ps)
                    nc.scalar.dma_start(out=o_d[bt, li], in_=ot)
    _hoist_excess_waits(nc)
    return nc


def _host_prep(x, Wd, Wu, Wl, Wtr, Wbl, np_dt):
    x = np.asarray(x, np.float32)
    Wd, Wu, Wl = np.asarray(Wd, np.float32), np.asarray(Wu, np.float32), np.asarray(Wl, np.float32)
    Wtr, Wbl = np.asarray(Wtr, np.float32), np.asarray(Wbl, np.float32)

    xT = np.ascontiguousarray(x.T)                       # [M*P, B]
    A = np.concatenate([Wtr[None], Wl], axis=0)          # weight applied to x_{i-1}
    Bst = Wd                                             # weight applied to x_i
    C = np.concatenate([Wu, Wbl[None]], axis=0)          # weight applied to x_{i+1}
    WT = np.stack([A, Bst, C], axis=1)                   # [64, 3, q, p]
    WT = np.ascontiguousarray(WT.transpose(0, 1, 3, 2))  # [64, 3, p, q]

    in_maps = []
    for c in range(NCORES):
        lo = (8 * c - 1) * P
        hi = (8 * c + 9) * P
        if lo < 0:
            xc = np.concatenate([xT[lo:], xT[:hi]], axis=0)
        elif hi > M * P:
            xc = np.concatenate([xT[lo:], xT[:hi - M * P]], axis=0)
        else:
            xc = xT[lo:hi]
        # partition-major slab [NBT, 128, NHALO*2, BT]: each batch-tile DMA
        # reads 20 KiB contiguous per partition instead of 1 KiB rows
        xs = xc.reshape(NHALO * 2, 128, NBT, BT).transpose(2, 1, 0, 3)
        xs = np.ascontiguousarray(xs, dtype=np_dt)
        wc = WT[8 * c:8 * c + 8].reshape(BPC, 3, 2, 128, P)
        wc = np.ascontiguousarray(
            wc.transpose(0, 3, 1, 2, 4).reshape(BPC, 128, 6, P), dtype=np_dt)
        in_maps.append({"xT": xs, "w": wc})
    return in_maps


def kernel(x, Wd, Wu, Wl, Wtr, Wbl):
    global LAST_EXEC_NS
    _apply_tile_patch()
    if TRACE:
        try:
            _install_profile_shim()
        except Exception as e:
            print(f"profile shim failed ({e}); running without trace")
    dt_in, np_dt = _DT[MODE]
    nc = _build_nc(MODE)
    in_maps = _host_prep(x, Wd, Wu, Wl, Wtr, Wbl, np_dt)
    res = run_bass_kernel_spmd(
        nc, in_maps, core_ids=list(range(NCORES)), trace=TRACE)
    LAST_EXEC_NS = res.exec_time_ns
    if TRACE and REPEATS > 1:
        global ALL_EXEC_NS
        ALL_EXEC_NS = [res.exec_time_ns]
        for _ in range(REPEATS - 1):
            r2 = run_bass_kernel_spmd(
                nc, in_maps, core_ids=list(range(NCORES)), trace=True)
            ALL_EXEC_NS.append(r2.exec_time_ns)
        LAST_EXEC_NS = min(t for t in ALL_EXEC_NS if t)
    # o slab per core: [NBT, BPC, 128, 2, BT] -> [BPC*P, B]
    outT = np.concatenate(
        [np.asarray(res.results[c]["o"]).transpose(1, 3, 2, 0, 4)
         .reshape(BPC * P, B) for c in range(NCORES)], axis=0)
    outT = np.asarray(outT, dtype=np.float32)
    return np.ascontiguousarray(outT.T)                  # [B, M*P] float32



# revision 27
# speedup vs baseline: 1.0118x; 1.0118x over previous
"""BlockXDiag (tri-diagonal block matrix × batch, periodic corners) on 8
Trainium2 NeuronCores.

Math (per reference): out_i = x_{i-1} @ A_i.T + x_i @ Wd_i.T + x_{i+1} @ Wu_i.T
for block-rows i in [0, 64), block size P=256, batch B=4096, with periodic
corner terms (x_63 @ Wtr.T into out_0, x_0 @ Wbl.T into out_63).

Sharding: output block-rows are split 8-per-core (weights split across
cores, which keeps per-core weight traffic small and lets each core stream
only its 10-block x halo). Everything device-side runs in bf16 (inputs cast
on host, output staged bf16 and upcast on host) so the kernel is
tensor-engine-bound: 768 matmuls/core x 216 ns = 166 us vs a 41 MB/core
HBM footprint (~115 us). Inputs are staged host-side as partition-major
slabs per batch-tile so every DMA is 20 KiB-contiguous per partition.

Device kernel per core: out.T[q, b] tiles [128, 512] accumulated in PSUM
over 6 matmuls (3 source blocks x 2 k-halves); x loads on the SP HW DMA
queue, weight loads (per-li chunks) + output stores (per-li, 256 KB) on
the Activation HW DMA queue; ~44 junk matmuls on a zeroed tile pre-warm
the PE HAM clock gate so the real stream runs at 2.4 GHz from the start.
Measured: 184,890 ns (baseline 235,304/245,723 ns), rel err 3.5e-3.
"""
import numpy as np
import ml_dtypes

import concourse.bass as bass
import concourse.mybir as mybir
from concourse.tile import TileContext
from concourse.vector_clock import ScopedClock
from concourse.bass_utils import run_bass_kernel_spmd

M, P, B = 64, 256, 4096
NCORES = 8
BPC = M // NCORES          # output blocks per core: 8
NHALO = BPC + 2            # x blocks needed per core: 10
ROWS = NHALO * P           # x^T rows per core: 2560
BT = 512                   # batch-tile (matmul moving free dim)
NBT = B // BT              # 8

MODE = "bf16"              # "f32" | "f32r" | "bf16"
TRACE = False              # set by test harness to profile
REPEATS = 1                # extra timed executions (test harness only)
LAST_EXEC_NS = None
ALL_EXEC_NS = None

_DT = {
    "f32": (mybir.dt.float32, np.float32),
    "f32r": (mybir.dt.float32r, np.float32),
    "bf16": (mybir.dt.bfloat16, ml_dtypes.bfloat16),
}


# ---------------------------------------------------------------------------
# Workarounds for the pinned walrus build's 1-wait-per-instruction cap.
# Tile's tail drain stuffs every outstanding sem wait onto one Drain, and
# self-loading fp32/fp32r matmuls can carry >1 wait with no Ldweights to
# spill to. Split both across extra same-engine instructions.
def _patched_drain_and_barrier(self, tick_clock, wait_clock):
    drain_inst = self.nc.sync.drain()
    wait_clock.add_sem_waits(
        drain_inst.ins, ScopedClock({None: tick_clock.global_clock})
    )
    si = drain_inst.ins.sync_info
    waits = list(si.on_wait)
    if len(waits) > 1:
        drain_inst.ins.sync_info = mybir.SyncInfo(
            on_wait=[waits[0]], on_update=list(si.on_update)
        )
        for w in waits[1:]:
            d2 = self.nc.sync.drain()
            d2.ins.sync_info = mybir.SyncInfo(on_wait=[w], on_update=[])
    self.nc.all_engine_barrier()
    assert self.sems is not None
    popped = self.nc._tile_sem_poison_stack.pop()
    assert popped is self._sem_poison
    self.nc.clear_and_free_semaphores(list(self.sems.allocated().values()))
    self.nc.all_engine_barrier()


def _apply_tile_patch():
    TileContext._drain_and_barrier = _patched_drain_and_barrier


def _install_profile_shim():
    """Make trace=True work in this container: provide the missing
    antenv.axon_hooks module (NTFF capture via ctypes into libaxon_pjrt.so)
    and skip the bucket upload of artifacts."""
    import sys, types, ctypes, contextlib
    import concourse.bass_utils as bu
    bu.upload_artifacts = lambda tmpdir: tmpdir
    try:
        from antenv.axon_hooks import get_axon_ntff_profile_hook  # noqa
        return
    except ImportError:
        pass
    so_path = "/opt/axon/libaxon_pjrt.so"
    lib = ctypes.CDLL(so_path)
    if not hasattr(lib, "axon_start_nrt_profile"):
        return
    lib.axon_start_nrt_profile.argtypes = [
        ctypes.POINTER(ctypes.c_int64), ctypes.c_size_t]
    lib.axon_start_nrt_profile.restype = ctypes.c_int64
    lib.axon_stop_nrt_profile.argtypes = [ctypes.c_char_p]
    lib.axon_stop_nrt_profile.restype = ctypes.c_int64

    @contextlib.contextmanager
    def _hook(output_dir, device_ids):
        import jax
        jax.devices()
        if device_ids:
            ids = (ctypes.c_int64 * len(device_ids))(*device_ids)
            rc = lib.axon_start_nrt_profile(ids, len(device_ids))
        else:
            rc = lib.axon_start_nrt_profile(None, 0)
        if rc != 0:
            raise RuntimeError(f"axon_start_nrt_profile rc={rc}")
        try:
            yield
        finally:
            n = lib.axon_stop_nrt_profile(str(output_dir).encode())
            print(f"profile: {n} file(s) written to {output_dir}")

    mod = types.ModuleType("antenv.axon_hooks")
    mod.get_axon_ntff_profile_hook = lambda: _hook
    mod.set_axon_ntff_profile_hook = lambda h: None
    sys.modules["antenv.axon_hooks"] = mod
    import antenv
    antenv.axon_hooks = mod


def _hoist_excess_waits(nc):
    """Any non-EventSemaphore instruction may carry at most 1 sem wait on
    this walrus build; move extras onto inserted same-engine NoOps."""
    for fn in nc.m.functions:
        for bb in fn.blocks:
            insts = bb.instructions
            newlist = []
            changed = False
            for inst in insts:
                si = inst.sync_info
                cap = 2 if isinstance(inst, mybir.InstEventSemaphore) else 1
                if si is not None and len(si.on_wait) > cap:
                    waits = list(si.on_wait)
                    for i, w in enumerate(waits[cap:]):
                        newlist.append(mybir.InstNoOp(
                            name=f"{inst.name}_waitnop{i}",
                            engine=inst.engine,
                            bass_nofuse=True,
                            sync_info=mybir.SyncInfo(on_wait=[w], on_update=[]),
                        ))
                    inst.sync_info = mybir.SyncInfo(
                        on_wait=waits[:cap], on_update=list(si.on_update))
                    changed = True
                newlist.append(inst)
            if changed:
                insts.clear()
                insts.extend(newlist)


# ---------------------------------------------------------------------------
def _build_nc(mode):
    dt_in, _ = _DT[mode]
    f32 = mybir.dt.float32
    # Stage the output in bf16 (host upcasts): halves HBM write traffic;
    # quantization of the final result adds <2e-3 rel err vs the 2e-2 gate.
    dt_out = mybir.dt.bfloat16 if mode == "bf16" else f32
    nc = bass.Bass()
    # Inputs are pre-tiled host-side into partition-major slabs so every DMA
    # transfer is long-contiguous per partition (the naive x^T layout gave
    # 1 KB rows in bf16, which halved effective DMA throughput).
    xT_d = nc.dram_tensor("xT", [NBT, 128, NHALO * 2, BT], dt_in,
                          kind="ExternalInput")
    w_d = nc.dram_tensor("w", [BPC, 128, 6, P], dt_in, kind="ExternalInput")
    o_d = nc.dram_tensor("o", [NBT, BPC, 128, 2, BT], dt_out,
                         kind="ExternalOutput")

    with TileContext(nc) as tc:
        with tc.tile_pool(name="wpool", bufs=1) as wpool, \
             tc.tile_pool(name="xpool", bufs=3) as xpool, \
             tc.tile_pool(name="opool", bufs=6) as opool, \
             tc.tile_pool(name="pspool", bufs=8, space="PSUM") as pspool:
            # PE warm-up: junk matmuls on a zeroed scratch tile start the HAM
            # activity window ~2us before the first x chunk lands, so the
            # real MM stream runs at 2.4 GHz almost from its first issue.
            wa = wpool.tile([128, 128], dt_in, tag="warm")
            nc.gpsimd.memset(wa, 0)
            wps = pspool.tile([128, BT], f32, tag="ps")
            NWARM = 44
            for i in range(NWARM):
                nc.tensor.matmul(wps[0:64, 0:128], wa[:, 0:64], wa[:, 0:128],
                                 start=(i == 0), stop=(i == NWARM - 1))

            w_sb = wpool.tile([128, BPC * 3 * 2, P], dt_in)
            # Weights go on the Activation-engine HW DMA queue (so they do
            # not sit in front of the x stream on the SP queue), in per-li
            # chunks so li=k's weights land just ahead of its matmuls.
            for li in range(BPC):
                nc.scalar.dma_start(
                    out=w_sb[:, li * 6:(li + 1) * 6, :],
                    in_=w_d[li])

            for bt in range(NBT):
                xt = xpool.tile([128, NHALO * 2, BT], dt_in, tag="x")
                if bt == 0:
                    # head split: 1-block chunks so li=0 starts after ~0.25 MB
                    for c in range(NHALO):
                        nc.sync.dma_start(
                            out=xt[:, c * 2:(c + 1) * 2, :],
                            in_=xT_d[0][:, c * 2:(c + 1) * 2, :],
                        )
                elif bt <= 2:
                    # the head window is bandwidth-tight: chunk these tiles
                    # too so matmuls wait on single blocks, not 2.6 MB tiles
                    for c in range(NHALO // 2):
                        nc.sync.dma_start(
                            out=xt[:, c * 4:(c + 1) * 4, :],
                            in_=xT_d[bt][:, c * 4:(c + 1) * 4, :],
                        )
                else:
                    nc.sync.dma_start(out=xt, in_=xT_d[bt])
                for li in range(BPC):       # fine-grained 256 KB out stores
                    ot = opool.tile([128, 2, BT], dt_out, tag="o")
                    if bt == 0 and li < 2:
                        # head: qh-innermost so only 1 new x block is needed
                        # per 4 matmuls while the first chunks stream in
                        ps0 = pspool.tile([128, BT], f32, tag="ps")
                        ps1 = pspool.tile([128, BT], f32, tag="ps")
                        pss = (ps0, ps1)
                        for s in range(3):
                            for kh in range(2):
                                for qh in range(2):
                                    nc.tensor.matmul(
                                        pss[qh],
                                        w_sb[:, (li * 3 + s) * 2 + kh,
                                             qh * 128:(qh + 1) * 128],
                                        xt[:, (li + s) * 2 + kh, :],
                                        start=(s == 0 and kh == 0),
                                        stop=(s == 2 and kh == 1),
                                    )
                        for qh in range(2):
                            nc.vector.tensor_copy(
                                out=ot[:, qh, :], in_=pss[qh])
                    else:
                        for qh in range(2):
                            ps = pspool.tile([128, BT], f32, tag="ps")
                            for s in range(3):
                                for kh in range(2):
                                    nc.tensor.matmul(
                                        ps,
                                        w_sb[:, (li * 3 + s) * 2 + kh,
                                             qh * 128:(qh + 1) * 128],
                                        xt[:, (li + s) * 2 + kh, :],
                                        start=(s == 0 and kh == 0),
                                        stop=(s == 2 and kh == 1),
                                    )
                            nc.vector.tensor_copy(
                                out=ot[:, qh, :], in_=ps)
                    nc.scalar.dma_start(out=o_d[bt, li], in_=ot)
    _hoist_excess_waits(nc)
    return nc


def _host_prep(x, Wd, Wu, Wl, Wtr, Wbl, np_dt):
    x = np.asarray(x, np.float32)
    Wd, Wu, Wl = np.asarray(Wd, np.float32), np.asarray(Wu, np.float32), np.asarray(Wl, np.float32)
    Wtr, Wbl = np.asarray(Wtr, np.float32), np.asarray(Wbl, np.float32)

    xT = np.ascontiguousarray(x.T)                       # [M*P, B]
    A = np.concatenate([Wtr[None], Wl], axis=0)          # weight applied to x_{i-1}
    Bst = Wd                                             # weight applied to x_i
    C = np.concatenate([Wu, Wbl[None]], axis=0)          # weight applied to x_{i+1}
    WT = np.stack([A, Bst, C], axis=1)                   # [64, 3, q, p]
    WT = np.ascontiguousarray(WT.transpose(0, 1, 3, 2))  # [64, 3, p, q]

    in_maps = []
    for c in range(NCORES):
        lo = (8 * c - 1) * P
        hi = (8 * c + 9) * P
        if lo < 0:
            xc = np.concatenate([xT[lo:], xT[:hi]], axis=0)
        elif hi > M * P:
            xc = np.concatenate([xT[lo:], xT[:hi - M * P]], axis=0)
        else:
            xc = xT[lo:hi]
        # partition-major slab [NBT, 128, NHALO*2, BT]: each batch-tile DMA
        # reads 20 KiB contiguous per partition instead of 1 KiB rows
        xs = xc.reshape(NHALO * 2, 128, NBT, BT).transpose(2, 1, 0, 3)
        xs = np.ascontiguousarray(xs, dtype=np_dt)
        wc = WT[8 * c:8 * c + 8].reshape(BPC, 3, 2, 128, P)
        wc = np.ascontiguousarray(
            wc.transpose(0, 3, 1, 2, 4).reshape(BPC, 128, 6, P), dtype=np_dt)
        in_maps.append({"xT": xs, "w": wc})
    return in_maps


def kernel(x, Wd, Wu, Wl, Wtr, Wbl):
    global LAST_EXEC_NS
    _apply_tile_patch()
    if TRACE:
        try:
            _install_profile_shim()
        except Exception as e:
            print(f"profile shim failed ({e}); running without trace")
    dt_in, np_dt = _DT[MODE]
    nc = _build_nc(MODE)
    in_maps = _host_prep(x, Wd, Wu, Wl, Wtr, Wbl, np_dt)
    res = run_bass_kernel_spmd(
        nc, in_maps, core_ids=list(range(NCORES)), trace=TRACE)
    LAST_EXEC_NS = res.exec_time_ns
    if TRACE and REPEATS > 1:
        global ALL_EXEC_NS
        ALL_EXEC_NS = [res.exec_time_ns]
        for _ in range(REPEATS - 1):
            r2 = run_bass_kernel_spmd(
                nc, in_maps, core_ids=list(range(NCORES)), trace=True)
            ALL_EXEC_NS.append(r2.exec_time_ns)
        LAST_EXEC_NS = min(t for t in ALL_EXEC_NS if t)
    # o slab per core: [NBT, BPC, 128, 2, BT] -> [BPC*P, B]
    outT = np.concatenate(
        [np.asarray(res.results[c]["o"]).transpose(1, 3, 2, 0, 4)
         .reshape(BPC * P, B) for c in range(NCORES)], axis=0)
    outT = np.asarray(outT, dtype=np.float32)
    return np.ascontiguousarray(outT.T)                  # [B, M*P] float32

